# revision 30
# baseline (speedup 1.0000x reference)
"""ODE-RNN Trainium2 kernel (latency-minimized serial chain).

Math (matches jax reference):
  per step t (times from batch[0,:,0], shared across batch):
    hp = ODE-integrate dh/dt = tanh(h @ A) over [t_prev, t], midpoint RK2:
         k1 = f(h), k2 = f(h + dt/2 k1), hp = h + dt k2   (A = W1.T @ W2.T)
    gru: r = sig(gi_r + gh_r), zc = 1 - sig(gi_z + gh_z) [z-weights negated],
         n = tanh(gi_n + r*gh_n)
    h' = hp + w*n - w*hp,  w = m*zc
  Gate pre-acts use the Euler state hpE = h + dt*k1 (adds ~0.7e-3 rel vs the
  2e-2 tolerance) so they don't wait for k2.

The T=64 recurrence is pure cross-engine latency; every engine is <45% busy.
Critical chain per step (everything else overlaps off-chain):
  [4MM U+=A@wn] -> [ACT k1] -> [DVE hpE] -> [4MM r-gates] -> [ACT sig r] ->
  [DVE scan: argn = r*gh + gi at odd slots] -> [ACT tanh n] -> [DVE wn=w*n]
Off-chain: stage-2 ((dt/2)A@k1 from the per-step a1s copies) + ACT k2 + the
sig_z/gpsimd w,wh,hw1 path; the state fold-in runs on the PE as
U' = A@hp + (-A)@wh + A@wn so no h-assembly op sits between the gates and
stage 1. Gate banks are PSUM-seeded by fold matmuls (exact fp16 hi/lo
splits; gi_n interleaved at odd slots; z negated so one sigmoid yields
[r|1-z]) and the n-gate product+bias-add is a single tensor_tensor_scan
over the [gh|gi]-interleaved PSUM bank.
"""
import numpy as np

import concourse.bass as bass
import concourse.bacc as bacc
import concourse.tile as tile
from concourse import mybir
from concourse.bass_utils import run_bass_kernel_spmd

B, T, H, D = 256, 64, 256, 512
NCORES = 8
BL = B // NCORES          # 32 batch rows per core
KT = H // 128             # 2 contraction tiles
F32 = mybir.dt.float32
F16 = mybir.dt.float16
AF = mybir.ActivationFunctionType
OP = mybir.AluOpType


def _build_program(dts, repeat=1):
    nc = bacc.Bacc(None, target_bir_lowering=False)

    a_d = nc.dram_tensor("a16", [128, KT * H], F16, kind="ExternalInput")
    na_d = nc.dram_tensor("nega16", [128, KT * H], F16, kind="ExternalInput")
    a1_d = nc.dram_tensor("a1s", [128, T, KT * H], F16, kind="ExternalInput")
    whh_d = nc.dram_tensor("whh16", [128, KT, 3 * H], F16, kind="ExternalInput")
    fw_d = nc.dram_tensor("foldw", [46, 128], F16, kind="ExternalInput")
    fx_d = nc.dram_tensor("foldx", [46, T, 128], F16, kind="ExternalInput")
    mrow_d = nc.dram_tensor("mrow", [1, T * BL], F16, kind="ExternalInput")
    out_d = nc.dram_tensor("h_out", [KT, 128, BL], F16, kind="ExternalOutput")

    with tile.TileContext(nc) as tc:
        with (
            tc.tile_pool(name="const", bufs=1) as const,
            tc.tile_pool(name="state", bufs=2) as state,
            tc.tile_pool(name="tmp", bufs=2) as tmp,
            tc.tile_pool(name="ps_u", bufs=2, space="PSUM") as ps_u,
            tc.tile_pool(name="ps_rz", bufs=2, space="PSUM") as ps_rz,
            tc.tile_pool(name="ps_n", bufs=2, space="PSUM") as ps_n,
        ):
            # ---- preload constants ----
            a_sb = const.tile([128, KT * H], F16)
            nc.sync.dma_start(out=a_sb, in_=a_d[:, :])
            na_sb = const.tile([128, KT * H], F16)
            nc.sync.dma_start(out=na_sb, in_=na_d[:, :])
            a1_sb = const.tile([128, T, KT * H], F16)
            for t0 in range(0, T, 8):      # stay under 64KB/partition/desc
                nc.sync.dma_start(out=a1_sb[:, t0:t0 + 8, :],
                                  in_=a1_d[:, t0:t0 + 8, :])
            whh_sb = const.tile([128, KT, 3 * H], F16)
            nc.sync.dma_start(out=whh_sb, in_=whh_d[:, :, :])
            fw_sb = const.tile([46, 128], F16)
            nc.sync.dma_start(out=fw_sb, in_=fw_d[:, :])
            fx_sb = const.tile([46, T, 128], F16)
            nc.sync.dma_start(out=fx_sb, in_=fx_d[:, :, :])
            m_sb = const.tile([128, T * BL], F16)
            mrow_ap = mrow_d[0, :]
            nc.sync.dma_start(
                out=m_sb,
                in_=bass.AP(tensor=mrow_ap.tensor, offset=mrow_ap.offset,
                            ap=[[0, 128], [1, T * BL]]),
            )
            # r-interleave buffer: odd slots get sigmoid(r); even slots stay 0
            r2_sb = const.tile([128, KT, BL, 2], F32)
            nc.vector.memset(r2_sb, 0.0)

            def lhsT_of(sb, k, m):
                return sb[:, k * H + m * 128:k * H + (m + 1) * 128]

            def whh_lhsT(k, g):
                return whh_sb[:, k, g * 128:(g + 1) * 128]

            def flat2(ap):
                return bass.AP(tensor=ap.tensor, offset=ap.offset,
                               ap=[list(ap.ap[0]), [1, 2 * KT * BL]])

            def body():
                hcur = state.tile([128, KT, BL], F16, tag="hcur")
                nc.vector.memset(hcur, 0.0)
                wn = state.tile([128, KT, BL], F16, tag="wn")
                nc.vector.memset(wn, 0.0)
                hpn0 = state.tile([128, KT, BL], F16, tag="hpn")
                nc.vector.memset(hpn0, 0.0)
                wh0 = state.tile([128, KT, BL], F16, tag="wh")
                nc.vector.memset(wh0, 0.0)

                # step-0 seeds: U(0) = A@hpn0 + (-A)@wh0 (both zero) + folds
                ps_ut = ps_u.tile([128, KT, BL], F32, tag="u")
                for m in range(2):
                    for k in range(KT):
                        nc.tensor.matmul(ps_ut[:, m, :], lhsT_of(a_sb, k, m),
                                         hpn0[:, k, :],
                                         start=(m == 0 and k == 0), stop=False,
                                         skip_group_check=True)
                for m in range(2):
                    for k in range(KT):
                        nc.tensor.matmul(ps_ut[:, m, :], lhsT_of(na_sb, k, m),
                                         wh0[:, k, :], start=False, stop=False,
                                         skip_group_check=True)
                ps_rzt = ps_rz.tile([128, 4, BL], F32, tag="rz")
                nc.tensor.matmul(ps_rzt[:, :, :], fw_sb[0:20, :],
                                 fx_sb[0:20, 0, :],
                                 start=True, stop=False, skip_group_check=True)
                ps_nt = ps_n.tile([128, KT, BL, 2], F32, tag="n")
                nc.tensor.matmul(ps_nt[:, :, :, :], fw_sb[32:46, :],
                                 fx_sb[32:46, 0, :],
                                 start=True, stop=False, skip_group_check=True)

                for t in range(T):
                    dt = float(dts[t])
                    a1t = a1_sb[:, t, :]

                    # ---- chain: finalize U with A@wn ----
                    for m in range(2):
                        for k in range(KT):
                            nc.tensor.matmul(ps_ut[:, m, :],
                                             lhsT_of(a_sb, k, m),
                                             wn[:, k, :], start=False,
                                             stop=False, skip_group_check=True)
                    # r-gates use the stale state hcur: their MMs and sig_r
                    # run in the ACT-idle window before k1, freeing an ACT
                    # slot ahead of k2 (state-fold path). Adds ~9e-3 rel,
                    # still 1.6x inside the 2e-2 gate.
                    for g in range(2):
                        for k in range(KT):
                            nc.tensor.matmul(ps_rzt[:, g, :], whh_lhsT(k, g),
                                             hcur[:, k, :], start=False,
                                             stop=False,
                                             skip_group_check=True)
                    # sigma_r into odd slots of the pre-zeroed interleave buf
                    nc.scalar.activation(r2_sb[:, :, :, 1], ps_rzt[:, 0:2, :],
                                         AF.Sigmoid)
                    k1h = tmp.tile([128, KT, BL], F16, tag="k1h")
                    nc.scalar.activation(k1h, ps_ut, AF.Tanh)
                    # z/n gates use the Euler state hpE = hcur + dt*k1
                    # (carried ODE state hpn stays midpoint-exact)
                    hpE = tmp.tile([128, KT, BL], F16, tag="hpE")
                    nc.vector.scalar_tensor_tensor(hpE, k1h, dt, hcur,
                                                   op0=OP.mult, op1=OP.add)

                    # n-gates first: the scan (chain) waits on psN
                    for g in (4, 5):
                        for k in range(KT):
                            nc.tensor.matmul(ps_nt[:, g - 4, :, 0],
                                             whh_lhsT(k, g),
                                             hpE[:, k, :], start=False,
                                             stop=(g == 5 and k == KT - 1),
                                             skip_group_check=True)
                    for g in (2, 3):
                        for k in range(KT):
                            nc.tensor.matmul(ps_rzt[:, g, :], whh_lhsT(k, g),
                                             hpE[:, k, :], start=False,
                                             stop=(g == 3 and k == KT - 1),
                                             skip_group_check=True)
                    # stage 2 (off-chain): U += (dt/2)A @ k1
                    for m in range(2):
                        for k in range(KT):
                            nc.tensor.matmul(ps_ut[:, m, :], lhsT_of(a1t, k, m),
                                             k1h[:, k, :], start=False,
                                             stop=(m == 1 and k == KT - 1),
                                             skip_group_check=True)

                    zc = tmp.tile([128, KT, BL], F32, tag="zc")
                    nc.scalar.activation(zc, ps_rzt[:, 2:4, :], AF.Sigmoid)
                    k2h = tmp.tile([128, KT, BL], F16, tag="k2h")
                    nc.scalar.activation(k2h, ps_ut, AF.Tanh)

                    # argn at odd slots: scan state = (r2 * state) + psn
                    #   even j: 0*state + gh = gh ; odd j: r*gh + gi
                    argn2 = tmp.tile([128, KT, BL, 2], F32, tag="argn2")
                    nc.vector.tensor_tensor_scan(
                        flat2(argn2), flat2(r2_sb), flat2(ps_nt),
                        0.0, op0=OP.mult, op1=OP.add)
                    hpn_n = state.tile([128, KT, BL], F16, tag="hpn")
                    nc.vector.scalar_tensor_tensor(hpn_n, k2h, dt, hcur,
                                                   op0=OP.mult, op1=OP.add)

                    n16 = tmp.tile([128, KT, BL], F16, tag="n16")
                    nc.scalar.activation(n16, argn2[:, :, :, 1], AF.Tanh)

                    # gpsimd: w = m*zc ; wh = w*hpn ; hw1 = hpn - wh
                    m_slice = m_sb[:, t * BL:(t + 1) * BL]
                    m_ap = bass.AP(tensor=m_slice.tensor, offset=m_slice.offset,
                                   ap=[list(m_slice.ap[0]), [0, KT], [1, BL]])
                    w = tmp.tile([128, KT, BL], F16, tag="w")
                    nc.gpsimd.tensor_mul(w, zc, m_ap)
                    wh_n = state.tile([128, KT, BL], F16, tag="wh")
                    nc.gpsimd.tensor_mul(wh_n, w, hpn_n)
                    hw1_n = tmp.tile([128, KT, BL], F16, tag="hw1")
                    nc.gpsimd.tensor_sub(hw1_n, hpn_n, wh_n)

                    # next-step U seed (off-chain): A@hpn then -A@wh
                    ps_un = ps_u.tile([128, KT, BL], F32, tag="u")
                    for m in range(2):
                        for k in range(KT):
                            nc.tensor.matmul(ps_un[:, m, :], lhsT_of(a_sb, k, m),
                                             hpn_n[:, k, :],
                                             start=(m == 0 and k == 0),
                                             stop=False, skip_group_check=True)
                    # next-step gate-bank folds
                    ps_rzn = ps_rz.tile([128, 4, BL], F32, tag="rz")
                    tn = min(t + 1, T - 1)
                    nc.tensor.matmul(ps_rzn[:, :, :], fw_sb[0:20, :],
                                     fx_sb[0:20, tn, :],
                                     start=True, stop=False,
                                     skip_group_check=True)
                    ps_nn = ps_n.tile([128, KT, BL, 2], F32, tag="n")
                    nc.tensor.matmul(ps_nn[:, :, :, :], fw_sb[32:46, :],
                                     fx_sb[32:46, tn, :],
                                     start=True, stop=False,
                                     skip_group_check=True)
                    for m in range(2):
                        for k in range(KT):
                            nc.tensor.matmul(ps_un[:, m, :], lhsT_of(na_sb, k, m),
                                             wh_n[:, k, :], start=False,
                                             stop=False, skip_group_check=True)

                    # ---- chain tail: wn = w*n ----
                    wn_n = state.tile([128, KT, BL], F16, tag="wn")
                    nc.vector.tensor_mul(wn_n, w, n16)
                    hcur_n = state.tile([128, KT, BL], F16, tag="hcur")
                    nc.vector.tensor_add(hcur_n, hw1_n, wn_n)

                    hcur, wn = hcur_n, wn_n
                    ps_ut, ps_rzt, ps_nt = ps_un, ps_rzn, ps_nn

                return hcur

            if repeat == 1:
                hfin = body()
            else:
                with tc.For_i(0, repeat, 1):
                    hfin = body()

            for k in range(KT):
                nc.sync.dma_start(out=out_d[k, :, :], in_=hfin[:, k, :])

    nc.finalize()
    return nc


def _prepare_inputs(batch, mask, W1, b1, W2, b2, W_ih, b_ih, W_hh, b_hh):
    batch = np.asarray(batch, np.float32)
    mask = np.asarray(mask, np.float32)
    W1 = np.asarray(W1, np.float32); b1 = np.asarray(b1, np.float32)
    W2 = np.asarray(W2, np.float32); b2 = np.asarray(b2, np.float32)
    W_ih = np.asarray(W_ih, np.float32); b_ih = np.asarray(b_ih, np.float32)
    W_hh = np.asarray(W_hh, np.float32); b_hh = np.asarray(b_hh, np.float32)

    A = (W1.T.astype(np.float64) @ W2.T.astype(np.float64)).astype(np.float32)
    c = (b1.astype(np.float64) @ W2.T.astype(np.float64) + b2).astype(np.float32)
    assert np.abs(c).max() == 0.0, "nonzero ODE bias not wired"

    times = batch[0, :, 0].astype(np.float64)
    dts = np.diff(np.concatenate([[0.0], times]))

    def a_blocks(M, dtype=np.float16):   # [H, H] -> [128, KT*H] k-tile concat
        return np.ascontiguousarray(np.concatenate(
            [M[k * 128:(k + 1) * 128, :] for k in range(KT)], axis=1)).astype(dtype)

    a16 = a_blocks(A)
    nega16 = a_blocks(-A)
    a1s = np.ascontiguousarray(np.stack(
        [a_blocks((A.astype(np.float64) * (0.5 * d)).astype(np.float32))
         for d in dts]).transpose(1, 0, 2))              # [128,T,KT*H] fp16

    WhhT = np.ascontiguousarray(W_hh.T).astype(np.float64)
    WhhT[:, H:2 * H] *= -1.0                             # negated z-gate
    WhhT32 = WhhT.astype(np.float32)
    whh16 = np.ascontiguousarray(
        np.stack([WhhT32[k * 128:(k + 1) * 128, :] for k in range(KT)], axis=1)
    ).astype(np.float16)

    # rz fold weights: rows per reg {whi, wlo, whi, bhi, blo}; z negated
    bsum = b_ih + b_hh
    foldw = np.zeros((46, 128), np.float16)
    for reg in range(4):                                 # r0 r1 z0 z1
        sgn = 1.0 if reg < 2 else -1.0
        wslice = sgn * W_ih[reg * 128:(reg + 1) * 128, 0]
        whi = wslice.astype(np.float16)
        wlo = (wslice - whi.astype(np.float32)).astype(np.float16)
        bs = sgn * bsum[reg * 128:(reg + 1) * 128]
        bshi = bs.astype(np.float16)
        bslo = (bs - bshi.astype(np.float32)).astype(np.float16)
        base = reg * 5
        foldw[base + 0] = whi
        foldw[base + 1] = wlo
        foldw[base + 2] = whi
        foldw[base + 3] = bshi
        foldw[base + 4] = bslo
    # n-bank fold (K=14 at rows 32..45): even slots seed b_hh_n, odd slots
    # get gi_n = x*W_ih_n + b_ih_n via hi/lo splits
    for reg in range(2):
        base = 32 + reg * 7
        bn = b_hh[2 * H + reg * 128:2 * H + (reg + 1) * 128]
        bnhi = bn.astype(np.float16)
        bnlo = (bn - bnhi.astype(np.float32)).astype(np.float16)
        wn_ = W_ih[2 * H + reg * 128:2 * H + (reg + 1) * 128, 0]
        wnhi = wn_.astype(np.float16)
        wnlo = (wn_ - wnhi.astype(np.float32)).astype(np.float16)
        bi = b_ih[2 * H + reg * 128:2 * H + (reg + 1) * 128]
        bihi = bi.astype(np.float16)
        bilo = (bi - bihi.astype(np.float32)).astype(np.float16)
        foldw[base + 0] = bnhi
        foldw[base + 1] = bnlo
        foldw[base + 2] = wnhi
        foldw[base + 3] = wnlo
        foldw[base + 4] = wnhi
        foldw[base + 5] = bihi
        foldw[base + 6] = bilo

    xs = batch[:, :, 1]
    in_maps = []
    for ci in range(NCORES):
        bs_ = slice(ci * BL, (ci + 1) * BL)
        xs_c = xs[bs_].T                                 # [T, BL]
        xhi = xs_c.astype(np.float16)
        xlo = (xs_c - xhi.astype(np.float32)).astype(np.float16)
        foldx = np.zeros((46, T, 128), np.float16)
        fx_rz = foldx[:20].reshape(20, T, 4, BL)
        for reg in range(4):                             # col block per reg
            base = reg * 5
            fx_rz[base + 0, :, reg, :] = xhi
            fx_rz[base + 1, :, reg, :] = xhi
            fx_rz[base + 2, :, reg, :] = xlo
            fx_rz[base + 3, :, reg, :] = 1.0
            fx_rz[base + 4, :, reg, :] = 1.0
        fx_n = foldx[32:].reshape(14, T, KT, BL, 2)
        for reg in range(2):
            base = reg * 7
            fx_n[base + 0, :, reg, :, 0] = 1.0
            fx_n[base + 1, :, reg, :, 0] = 1.0
            fx_n[base + 2, :, reg, :, 1] = xhi
            fx_n[base + 3, :, reg, :, 1] = xhi
            fx_n[base + 4, :, reg, :, 1] = xlo
            fx_n[base + 5, :, reg, :, 1] = 1.0
            fx_n[base + 6, :, reg, :, 1] = 1.0
        mrow = np.ascontiguousarray(
            mask[bs_].T.reshape(1, -1)).astype(np.float16)
        im = {
            "a16": a16, "nega16": nega16, "a1s": a1s, "whh16": whh16,
            "foldw": foldw, "foldx": np.ascontiguousarray(foldx),
            "mrow": mrow,
        }
        in_maps.append(im)
    return dts, in_maps


def kernel(batch, mask, W1, b1, W2, b2, W_ih, b_ih, W_hh, b_hh):
    dts, in_maps = _prepare_inputs(batch, mask, W1, b1, W2, b2,
                                   W_ih, b_ih, W_hh, b_hh)
    nc = _build_program([float(d) for d in dts])
    res = run_bass_kernel_spmd(nc, in_maps, core_ids=list(range(NCORES)))

    out = np.empty((B, H), np.float32)
    for ci in range(NCORES):
        ho = res.results[ci]["h_out"]                    # [KT, 128, BL] fp16
        for k in range(KT):
            out[ci * BL:(ci + 1) * BL, k * 128:(k + 1) * 128] = \
                ho[k].astype(np.float32).T
    return out


# revision 31
# speedup vs baseline: 1.0706x; 1.0706x over previous
"""ODE-RNN Trainium2 kernel (latency-minimized serial chain).

Math (matches jax reference):
  per step t (times from batch[0,:,0], shared across batch):
    hp = ODE-integrate dh/dt = tanh(h @ A) over [t_prev, t], midpoint RK2:
         k1 = f(h), k2 = f(h + dt/2 k1), hp = h + dt k2   (A = W1.T @ W2.T)
    gru: r = sig(gi_r + gh_r), zc = 1 - sig(gi_z + gh_z) [z-weights negated],
         n = tanh(gi_n + r*gh_n)
    h' = hp + w*n - w*hp,  w = m*zc
  Gate pre-acts use the Euler state hpE = h + dt*k1 (adds ~0.7e-3 rel vs the
  2e-2 tolerance) so they don't wait for k2.

The T=64 recurrence is pure cross-engine latency; every engine is <45% busy.
Critical chain per step (everything else overlaps off-chain):
  [4MM U+=A@wn] -> [ACT k1] -> [DVE hpE] -> [4MM r-gates] -> [ACT sig r] ->
  [DVE scan: argn = r*gh + gi at odd slots] -> [ACT tanh n] -> [DVE wn=w*n]
Off-chain: stage-2 ((dt/2)A@k1 from the per-step a1s copies) + ACT k2 + the
sig_z/gpsimd w,wh,hw1 path; the state fold-in runs on the PE as
U' = A@hp + (-A)@wh + A@wn so no h-assembly op sits between the gates and
stage 1. Gate banks are PSUM-seeded by fold matmuls (exact fp16 hi/lo
splits; gi_n interleaved at odd slots; z negated so one sigmoid yields
[r|1-z]) and the n-gate product+bias-add is a single tensor_tensor_scan
over the [gh|gi]-interleaved PSUM bank.
"""
import numpy as np

import concourse.bass as bass
import concourse.bacc as bacc
import concourse.tile as tile
from concourse import mybir
from concourse.bass_utils import run_bass_kernel_spmd

B, T, H, D = 256, 64, 256, 512
NCORES = 8
BL = B // NCORES          # 32 batch rows per core
KT = H // 128             # 2 contraction tiles
F32 = mybir.dt.float32
F16 = mybir.dt.float16
AF = mybir.ActivationFunctionType
OP = mybir.AluOpType


def _build_program(dts, repeat=1):
    nc = bacc.Bacc(None, target_bir_lowering=False)

    a_d = nc.dram_tensor("a16", [128, KT * H], F16, kind="ExternalInput")
    na_d = nc.dram_tensor("nega16", [128, KT * H], F16, kind="ExternalInput")
    a1_d = nc.dram_tensor("a1s", [128, T, KT * H], F16, kind="ExternalInput")
    whh_d = nc.dram_tensor("whh16", [128, KT, 3 * H], F16, kind="ExternalInput")
    fw_d = nc.dram_tensor("foldw", [46, 128], F16, kind="ExternalInput")
    fx_d = nc.dram_tensor("foldx", [46, T, 128], F16, kind="ExternalInput")
    mrow_d = nc.dram_tensor("mrow", [1, T * BL], F16, kind="ExternalInput")
    out_d = nc.dram_tensor("h_out", [KT, 128, BL], F16, kind="ExternalOutput")

    with tile.TileContext(nc) as tc:
        with (
            tc.tile_pool(name="const", bufs=1) as const,
            tc.tile_pool(name="state", bufs=2) as state,
            tc.tile_pool(name="tmp", bufs=2) as tmp,
            tc.tile_pool(name="ps_u", bufs=2, space="PSUM") as ps_u,
            tc.tile_pool(name="ps_rz", bufs=2, space="PSUM") as ps_rz,
            tc.tile_pool(name="ps_n", bufs=2, space="PSUM") as ps_n,
        ):
            # ---- preload constants ----
            a_sb = const.tile([128, KT * H], F16)
            nc.sync.dma_start(out=a_sb, in_=a_d[:, :])
            na_sb = const.tile([128, KT * H], F16)
            nc.sync.dma_start(out=na_sb, in_=na_d[:, :])
            a1_sb = const.tile([128, T, KT * H], F16)
            for t0 in range(0, T, 8):      # stay under 64KB/partition/desc
                nc.sync.dma_start(out=a1_sb[:, t0:t0 + 8, :],
                                  in_=a1_d[:, t0:t0 + 8, :])
            whh_sb = const.tile([128, KT, 3 * H], F16)
            nc.sync.dma_start(out=whh_sb, in_=whh_d[:, :, :])
            fw_sb = const.tile([46, 128], F16)
            nc.sync.dma_start(out=fw_sb, in_=fw_d[:, :])
            fx_sb = const.tile([46, T, 128], F16)
            nc.sync.dma_start(out=fx_sb, in_=fx_d[:, :, :])
            m_sb = const.tile([128, T * BL], F16)
            mrow_ap = mrow_d[0, :]
            nc.sync.dma_start(
                out=m_sb,
                in_=bass.AP(tensor=mrow_ap.tensor, offset=mrow_ap.offset,
                            ap=[[0, 128], [1, T * BL]]),
            )
            # r-interleave buffer: odd slots get sigmoid(r); even slots stay 0
            r2_sb = const.tile([128, KT, BL, 2], F32)
            nc.vector.memset(r2_sb, 0.0)

            def lhsT_of(sb, k, m):
                return sb[:, k * H + m * 128:k * H + (m + 1) * 128]

            def whh_lhsT(k, g):
                return whh_sb[:, k, g * 128:(g + 1) * 128]

            def flat2(ap):
                return bass.AP(tensor=ap.tensor, offset=ap.offset,
                               ap=[list(ap.ap[0]), [1, 2 * KT * BL]])

            def body():
                hcur = state.tile([128, KT, BL], F16, tag="hcur")
                nc.vector.memset(hcur, 0.0)
                wn = state.tile([128, KT, BL], F16, tag="wn")
                nc.vector.memset(wn, 0.0)
                hpn0 = state.tile([128, KT, BL], F16, tag="hpn")
                nc.vector.memset(hpn0, 0.0)
                wh0 = state.tile([128, KT, BL], F16, tag="wh")
                nc.vector.memset(wh0, 0.0)

                # step-0 seeds: U(0) = A@hpn0 + (-A)@wh0 (both zero) + folds
                ps_ut = ps_u.tile([128, KT, BL], F32, tag="u")
                for m in range(2):
                    for k in range(KT):
                        nc.tensor.matmul(ps_ut[:, m, :], lhsT_of(a_sb, k, m),
                                         hpn0[:, k, :],
                                         start=(m == 0 and k == 0), stop=False,
                                         skip_group_check=True)
                for m in range(2):
                    for k in range(KT):
                        nc.tensor.matmul(ps_ut[:, m, :], lhsT_of(na_sb, k, m),
                                         wh0[:, k, :], start=False, stop=False,
                                         skip_group_check=True)
                ps_rzt = ps_rz.tile([128, 4, BL], F32, tag="rz")
                nc.tensor.matmul(ps_rzt[:, :, :], fw_sb[0:20, :],
                                 fx_sb[0:20, 0, :],
                                 start=True, stop=False, skip_group_check=True)
                ps_nt = ps_n.tile([128, KT, BL, 2], F32, tag="n")
                nc.tensor.matmul(ps_nt[:, :, :, :], fw_sb[32:46, :],
                                 fx_sb[32:46, 0, :],
                                 start=True, stop=False, skip_group_check=True)

                for t in range(T):
                    dt = float(dts[t])
                    a1t = a1_sb[:, t, :]

                    # ---- chain: finalize U with A@wn ----
                    for m in range(2):
                        for k in range(KT):
                            nc.tensor.matmul(ps_ut[:, m, :],
                                             lhsT_of(a_sb, k, m),
                                             wn[:, k, :], start=False,
                                             stop=False, skip_group_check=True)
                    k1h = tmp.tile([128, KT, BL], F16, tag="k1h")
                    nc.scalar.activation(k1h, ps_ut, AF.Tanh)
                    # gates use the Euler state hpE = hcur + dt*k1 (carried
                    # ODE state hpn stays midpoint-exact)
                    hpE = tmp.tile([128, KT, BL], F16, tag="hpE")
                    nc.vector.scalar_tensor_tensor(hpE, k1h, dt, hcur,
                                                   op0=OP.mult, op1=OP.add)

                    # ---- GRU matmuls on hpE (r-gates first: sig_r on-chain)
                    for g in range(2):
                        for k in range(KT):
                            nc.tensor.matmul(ps_rzt[:, g, :], whh_lhsT(k, g),
                                             hpE[:, k, :], start=False,
                                             stop=False,
                                             skip_group_check=True)
                    for g in (2, 3):
                        for k in range(KT):
                            nc.tensor.matmul(ps_rzt[:, g, :], whh_lhsT(k, g),
                                             hpE[:, k, :], start=False,
                                             stop=(g == 3 and k == KT - 1),
                                             skip_group_check=True)
                    for g in (4, 5):
                        for k in range(KT):
                            nc.tensor.matmul(ps_nt[:, g - 4, :, 0],
                                             whh_lhsT(k, g),
                                             hpE[:, k, :], start=False,
                                             stop=(g == 5 and k == KT - 1),
                                             skip_group_check=True)
                    # stage 2 (off-chain): U += (dt/2)A @ k1
                    for m in range(2):
                        for k in range(KT):
                            nc.tensor.matmul(ps_ut[:, m, :], lhsT_of(a1t, k, m),
                                             k1h[:, k, :], start=False,
                                             stop=(m == 1 and k == KT - 1),
                                             skip_group_check=True)

                    # sigma_r into odd slots of the pre-zeroed interleave buf
                    nc.scalar.activation(r2_sb[:, :, :, 1], ps_rzt[:, 0:2, :],
                                         AF.Sigmoid)
                    zc = tmp.tile([128, KT, BL], F32, tag="zc")
                    nc.scalar.activation(zc, ps_rzt[:, 2:4, :], AF.Sigmoid)
                    k2h = tmp.tile([128, KT, BL], F16, tag="k2h")
                    nc.scalar.activation(k2h, ps_ut, AF.Tanh)

                    # argn at odd slots: scan state = (r2 * state) + psn
                    #   even j: 0*state + gh = gh ; odd j: r*gh + gi
                    argn2 = tmp.tile([128, KT, BL, 2], F32, tag="argn2")
                    nc.vector.tensor_tensor_scan(
                        flat2(argn2), flat2(r2_sb), flat2(ps_nt),
                        0.0, op0=OP.mult, op1=OP.add)
                    hpn_n = state.tile([128, KT, BL], F16, tag="hpn")
                    nc.vector.scalar_tensor_tensor(hpn_n, k2h, dt, hcur,
                                                   op0=OP.mult, op1=OP.add)

                    n16 = tmp.tile([128, KT, BL], F16, tag="n16")
                    nc.scalar.activation(n16, argn2[:, :, :, 1], AF.Tanh)

                    # gpsimd: w = m*zc ; wh = w*hpn ; hw1 = hpn - wh
                    m_slice = m_sb[:, t * BL:(t + 1) * BL]
                    m_ap = bass.AP(tensor=m_slice.tensor, offset=m_slice.offset,
                                   ap=[list(m_slice.ap[0]), [0, KT], [1, BL]])
                    w = tmp.tile([128, KT, BL], F16, tag="w")
                    nc.gpsimd.tensor_mul(w, zc, m_ap)
                    wh_n = state.tile([128, KT, BL], F16, tag="wh")
                    nc.gpsimd.tensor_mul(wh_n, w, hpn_n)
                    hw1_n = tmp.tile([128, KT, BL], F16, tag="hw1")
                    nc.gpsimd.tensor_sub(hw1_n, hpn_n, wh_n)

                    # next-step U seed (off-chain): A@hpn then -A@wh
                    ps_un = ps_u.tile([128, KT, BL], F32, tag="u")
                    for m in range(2):
                        for k in range(KT):
                            nc.tensor.matmul(ps_un[:, m, :], lhsT_of(a_sb, k, m),
                                             hpn_n[:, k, :],
                                             start=(m == 0 and k == 0),
                                             stop=False, skip_group_check=True)
                    # next-step gate-bank folds
                    ps_rzn = ps_rz.tile([128, 4, BL], F32, tag="rz")
                    tn = min(t + 1, T - 1)
                    nc.tensor.matmul(ps_rzn[:, :, :], fw_sb[0:20, :],
                                     fx_sb[0:20, tn, :],
                                     start=True, stop=False,
                                     skip_group_check=True)
                    ps_nn = ps_n.tile([128, KT, BL, 2], F32, tag="n")
                    nc.tensor.matmul(ps_nn[:, :, :, :], fw_sb[32:46, :],
                                     fx_sb[32:46, tn, :],
                                     start=True, stop=False,
                                     skip_group_check=True)
                    for m in range(2):
                        for k in range(KT):
                            nc.tensor.matmul(ps_un[:, m, :], lhsT_of(na_sb, k, m),
                                             wh_n[:, k, :], start=False,
                                             stop=False, skip_group_check=True)

                    # ---- chain tail: wn = w*n ----
                    wn_n = state.tile([128, KT, BL], F16, tag="wn")
                    nc.vector.tensor_mul(wn_n, w, n16)
                    hcur_n = state.tile([128, KT, BL], F16, tag="hcur")
                    nc.vector.tensor_add(hcur_n, hw1_n, wn_n)

                    hcur, wn = hcur_n, wn_n
                    ps_ut, ps_rzt, ps_nt = ps_un, ps_rzn, ps_nn

                return hcur

            if repeat == 1:
                hfin = body()
            else:
                with tc.For_i(0, repeat, 1):
                    hfin = body()

            for k in range(KT):
                nc.sync.dma_start(out=out_d[k, :, :], in_=hfin[:, k, :])

    nc.finalize()
    return nc


def _prepare_inputs(batch, mask, W1, b1, W2, b2, W_ih, b_ih, W_hh, b_hh):
    batch = np.asarray(batch, np.float32)
    mask = np.asarray(mask, np.float32)
    W1 = np.asarray(W1, np.float32); b1 = np.asarray(b1, np.float32)
    W2 = np.asarray(W2, np.float32); b2 = np.asarray(b2, np.float32)
    W_ih = np.asarray(W_ih, np.float32); b_ih = np.asarray(b_ih, np.float32)
    W_hh = np.asarray(W_hh, np.float32); b_hh = np.asarray(b_hh, np.float32)

    A = (W1.T.astype(np.float64) @ W2.T.astype(np.float64)).astype(np.float32)
    c = (b1.astype(np.float64) @ W2.T.astype(np.float64) + b2).astype(np.float32)
    assert np.abs(c).max() == 0.0, "nonzero ODE bias not wired"

    times = batch[0, :, 0].astype(np.float64)
    dts = np.diff(np.concatenate([[0.0], times]))

    def a_blocks(M, dtype=np.float16):   # [H, H] -> [128, KT*H] k-tile concat
        return np.ascontiguousarray(np.concatenate(
            [M[k * 128:(k + 1) * 128, :] for k in range(KT)], axis=1)).astype(dtype)

    a16 = a_blocks(A)
    nega16 = a_blocks(-A)
    a1s = np.ascontiguousarray(np.stack(
        [a_blocks((A.astype(np.float64) * (0.5 * d)).astype(np.float32))
         for d in dts]).transpose(1, 0, 2))              # [128,T,KT*H] fp16

    WhhT = np.ascontiguousarray(W_hh.T).astype(np.float64)
    WhhT[:, H:2 * H] *= -1.0                             # negated z-gate
    WhhT32 = WhhT.astype(np.float32)
    whh16 = np.ascontiguousarray(
        np.stack([WhhT32[k * 128:(k + 1) * 128, :] for k in range(KT)], axis=1)
    ).astype(np.float16)

    # rz fold weights: rows per reg {whi, wlo, whi, bhi, blo}; z negated
    bsum = b_ih + b_hh
    foldw = np.zeros((46, 128), np.float16)
    for reg in range(4):                                 # r0 r1 z0 z1
        sgn = 1.0 if reg < 2 else -1.0
        wslice = sgn * W_ih[reg * 128:(reg + 1) * 128, 0]
        whi = wslice.astype(np.float16)
        wlo = (wslice - whi.astype(np.float32)).astype(np.float16)
        bs = sgn * bsum[reg * 128:(reg + 1) * 128]
        bshi = bs.astype(np.float16)
        bslo = (bs - bshi.astype(np.float32)).astype(np.float16)
        base = reg * 5
        foldw[base + 0] = whi
        foldw[base + 1] = wlo
        foldw[base + 2] = whi
        foldw[base + 3] = bshi
        foldw[base + 4] = bslo
    # n-bank fold (K=14 at rows 32..45): even slots seed b_hh_n, odd slots
    # get gi_n = x*W_ih_n + b_ih_n via hi/lo splits
    for reg in range(2):
        base = 32 + reg * 7
        bn = b_hh[2 * H + reg * 128:2 * H + (reg + 1) * 128]
        bnhi = bn.astype(np.float16)
        bnlo = (bn - bnhi.astype(np.float32)).astype(np.float16)
        wn_ = W_ih[2 * H + reg * 128:2 * H + (reg + 1) * 128, 0]
        wnhi = wn_.astype(np.float16)
        wnlo = (wn_ - wnhi.astype(np.float32)).astype(np.float16)
        bi = b_ih[2 * H + reg * 128:2 * H + (reg + 1) * 128]
        bihi = bi.astype(np.float16)
        bilo = (bi - bihi.astype(np.float32)).astype(np.float16)
        foldw[base + 0] = bnhi
        foldw[base + 1] = bnlo
        foldw[base + 2] = wnhi
        foldw[base + 3] = wnlo
        foldw[base + 4] = wnhi
        foldw[base + 5] = bihi
        foldw[base + 6] = bilo

    xs = batch[:, :, 1]
    in_maps = []
    for ci in range(NCORES):
        bs_ = slice(ci * BL, (ci + 1) * BL)
        xs_c = xs[bs_].T                                 # [T, BL]
        xhi = xs_c.astype(np.float16)
        xlo = (xs_c - xhi.astype(np.float32)).astype(np.float16)
        foldx = np.zeros((46, T, 128), np.float16)
        fx_rz = foldx[:20].reshape(20, T, 4, BL)
        for reg in range(4):                             # col block per reg
            base = reg * 5
            fx_rz[base + 0, :, reg, :] = xhi
            fx_rz[base + 1, :, reg, :] = xhi
            fx_rz[base + 2, :, reg, :] = xlo
            fx_rz[base + 3, :, reg, :] = 1.0
            fx_rz[base + 4, :, reg, :] = 1.0
        fx_n = foldx[32:].reshape(14, T, KT, BL, 2)
        for reg in range(2):
            base = reg * 7
            fx_n[base + 0, :, reg, :, 0] = 1.0
            fx_n[base + 1, :, reg, :, 0] = 1.0
            fx_n[base + 2, :, reg, :, 1] = xhi
            fx_n[base + 3, :, reg, :, 1] = xhi
            fx_n[base + 4, :, reg, :, 1] = xlo
            fx_n[base + 5, :, reg, :, 1] = 1.0
            fx_n[base + 6, :, reg, :, 1] = 1.0
        mrow = np.ascontiguousarray(
            mask[bs_].T.reshape(1, -1)).astype(np.float16)
        im = {
            "a16": a16, "nega16": nega16, "a1s": a1s, "whh16": whh16,
            "foldw": foldw, "foldx": np.ascontiguousarray(foldx),
            "mrow": mrow,
        }
        in_maps.append(im)
    return dts, in_maps


def kernel(batch, mask, W1, b1, W2, b2, W_ih, b_ih, W_hh, b_hh):
    dts, in_maps = _prepare_inputs(batch, mask, W1, b1, W2, b2,
                                   W_ih, b_ih, W_hh, b_hh)
    nc = _build_program([float(d) for d in dts])
    res = run_bass_kernel_spmd(nc, in_maps, core_ids=list(range(NCORES)))

    out = np.empty((B, H), np.float32)
    for ci in range(NCORES):
        ho = res.results[ci]["h_out"]                    # [KT, 128, BL] fp16
        for k in range(KT):
            out[ci * BL:(ci + 1) * BL, k * 128:(k + 1) * 128] = \
                ho[k].astype(np.float32).T
    return out
